# revision 29
# baseline (speedup 1.0000x reference)
"""Bilinear pooling kernel for 8 Trainium2 NeuronCores (Bass/Tile).

Computes out[b,n,v,o] = sum_{d,e} node[b,n,d] * veh[b,v,e] * W[o, d*E+e] + bias[o]
for B=16, N=64, V=16, D=E=128, O=256.

Strategy: tensor-shard over the output dim O (32 channels per core, no
communication). Per core, two matmul stages:
  Stage A:  U[d, (b,v,o)] = sum_e W3[o,d,e] * veh[b,v,e]
            32 matmuls: lhsT = W3[o].T [e=128, d=128], rhs = vehT [e=128, (b,v)=256]
  Stage B:  out[b][n, (v,o)] = sum_d node[b,n,d] * U[d, b, v, o]
            16 matmuls: lhsT = nodeT_b [d=128, n=64], rhs = U_b [d=128, (v,o)=512]
The host concatenates the 8 per-core [B,N,V,32] outputs along the last
axis and adds the bias during the unshard.
"""

import os
import sys

import numpy as np

sys.path.insert(0, "/opt/trn_rl_repo")

B, N, V = 16, 64, 16
D = 128
E = 128
O = 256
NCORES = 8
OS = O // NCORES  # 32 output channels per core
VO = V * OS  # 512

# f32  : plain fp32 matmuls (exact, 4 cycles/row)
# f32r : fp32 data in reduced-precision streaming mode (~2x faster than f32)
# bf16 : inputs cast to bf16 on host, full-rate matmuls
MODE = os.environ.get("BILIN_MODE", "bf16")

_nc_cache = {}


def _build(mode):
    from contextlib import ExitStack

    import concourse.tile as tile
    from concourse import bacc, mybir

    f32 = mybir.dt.float32
    if mode == "bf16":
        mmdt = mybir.dt.bfloat16
    elif mode == "f32r":
        mmdt = mybir.dt.float32r
    else:
        mmdt = f32

    nc = bacc.Bacc("TRN2", target_bir_lowering=False)
    nodeT_d = nc.dram_tensor("nodeT", [D, B * N], mmdt, kind="ExternalInput")
    vehT_d = nc.dram_tensor("vehT", [E, B * V], mmdt, kind="ExternalInput")
    wt_d = nc.dram_tensor("wt", [E, OS * D], mmdt, kind="ExternalInput")
    # n-major output so one [64, 2, 512] SBUF tile flushes as one DMA
    out_d = nc.dram_tensor("out", [N, B, VO], f32, kind="ExternalOutput")

    # Input DMAs live in a raw block BEFORE the TileContext so the
    # transfers stream in during the engine-startup preamble (~6us)
    # instead of after it. Completion is signalled via per-ring
    # semaphores; each ring is FIFO so one cumulative threshold per
    # chunk suffices, and the PE queue is FIFO so only the first
    # consumer matmul of each chunk needs the wait.
    vehT_sb = nc.alloc_sbuf_tensor("vehT_sb", [E, B * V], mmdt)
    nodeT_sb = nc.alloc_sbuf_tensor("nodeT_sb", [D, B * N], mmdt)
    # (o_start, n_channels, ring); per-ring issue order == list order
    WSPLIT = [
        (0, 3, "sync"), (3, 6, "scalar"), (9, 7, "scalar"),
        (16, 8, "sync"), (24, 8, "gpsimd"),
    ]
    wt_sb = [
        nc.alloc_sbuf_tensor(f"wt_sb{k}", [E, no * D], mmdt)
        for k, (o0, no, en) in enumerate(WSPLIT)
    ]
    sems = {
        "sync": nc.alloc_semaphore("dma_sync"),
        "scalar": nc.alloc_semaphore("dma_scalar"),
        "gpsimd": nc.alloc_semaphore("dma_gpsimd"),
    }
    warm_sem = nc.alloc_semaphore("warm_sem")
    warm_sb = nc.alloc_sbuf_tensor("warm_sb", [D, B * V], mmdt)
    ring_total = {"sync": 0, "scalar": 0, "gpsimd": 0}
    ring_total["sync"] += 16  # vehT
    for o0, no, en in WSPLIT:
        ring_total[en] += 16
    ring_total["gpsimd"] += 16  # nodeT

    def wait_all_rings(eng):
        eng.wait_ge(sems["sync"], ring_total["sync"])
        eng.wait_ge(sems["scalar"], ring_total["scalar"])
        eng.wait_ge(sems["gpsimd"], ring_total["gpsimd"])

    # Pre-TileContext block: input DMAs stream in and the PE clock-gate
    # warms up (dummy matmuls on zeroed SBUF) while the engines start up;
    # every engine then waits for all three DMA rings, which doubles as
    # the input barrier for the Tile block.
    from contextlib import ExitStack as _ES

    with nc.psum_tensor("warm_ps", [D, B * V], f32) as warm_ps, nc.Block() as input_block:

        @input_block.sync
        def _(sync):
            sync.dma_start(vehT_sb[:], vehT_d[:]).then_inc(sems["sync"], 16)
            for k, (o0, no, en) in enumerate(WSPLIT):
                if en == "sync":
                    sync.dma_start(
                        wt_sb[k][:], wt_d[:, o0 * D : (o0 + no) * D]
                    ).then_inc(sems["sync"], 16)
            wait_all_rings(sync)

        @input_block.scalar
        def _(scalar):
            for k, (o0, no, en) in enumerate(WSPLIT):
                if en == "scalar":
                    scalar.dma_start(
                        wt_sb[k][:], wt_d[:, o0 * D : (o0 + no) * D]
                    ).then_inc(sems["scalar"], 16)
            wait_all_rings(scalar)

        @input_block.gpsimd
        def _(gpsimd):
            for k, (o0, no, en) in enumerate(WSPLIT):
                if en == "gpsimd":
                    gpsimd.dma_start(
                        wt_sb[k][:], wt_d[:, o0 * D : (o0 + no) * D]
                    ).then_inc(sems["gpsimd"], 16)
            gpsimd.dma_start(nodeT_sb[:], nodeT_d[:]).then_inc(
                sems["gpsimd"], 16
            )
            wait_all_rings(gpsimd)

        @input_block.vector
        def _(vector):
            vector.memset(warm_sb[:], 0).then_inc(warm_sem, 1)
            wait_all_rings(vector)

        @input_block.tensor
        def _(tensor):
            tensor.wait_ge(warm_sem, 1)
            for i in range(18):
                tensor.matmul(
                    warm_ps[:, 0 : B * V], warm_sb[:, 0:D], warm_sb[:],
                    start=True, stop=True,
                )
            wait_all_rings(tensor)

    with ExitStack() as ctx:
        tc = ctx.enter_context(tile.TileContext(nc))
        const = ctx.enter_context(tc.tile_pool(name="const", bufs=1))
        upool = ctx.enter_context(tc.tile_pool(name="u", bufs=1))
        psum = ctx.enter_context(tc.tile_pool(name="psum", bufs=4, space="PSUM"))
        outp = ctx.enter_context(tc.tile_pool(name="outp", bufs=8))

        vehT = vehT_sb[:]
        nodeT = nodeT_sb[:]

        def wsel(o):
            for k, (o0, no, en) in enumerate(WSPLIT):
                if o0 <= o < o0 + no:
                    return wt_sb[k][:, (o - o0) * D : (o - o0 + 1) * D], k
            raise AssertionError(o)

        # U[d, o, b, v] staged in SBUF for stage B (o-major so the stage-A
        # PSUM evacuation is one contiguous copy per psum tile)
        U = upool.tile([D, OS, B, V], mmdt)

        # Stage A: 8 psum tiles of [128, 4, 256] (2 banks, 4 o-channels)
        for g in range(OS // 4):
            pa = psum.tile([D, 4, B, V], f32, tag="ps")
            for i in range(4):
                o = 4 * g + i
                w_ap, k = wsel(o)
                nc.tensor.matmul(
                    pa[:, i], w_ap, vehT, start=True, stop=True,
                )
            if g % 2 == 0:
                nc.vector.tensor_copy(U[:, 4 * g : 4 * g + 4, :, :], pa[:])
            else:
                nc.scalar.copy(U[:, 4 * g : 4 * g + 4, :, :], pa[:])

        # Stage B: psum tiles [64, 2, 512] (2 banks, 2 batches); bias is
        # added on the host during unshard
        for p in range(B // 2):
            b0, b1 = 2 * p, 2 * p + 1
            pb = psum.tile([N, 2, VO], f32, tag="ps")
            nc.tensor.matmul(
                pb[:, 0], nodeT[:, b0 * N : (b0 + 1) * N], U[:, :, b0, :],
                start=True, stop=True,
            )
            nc.tensor.matmul(
                pb[:, 1], nodeT[:, b1 * N : (b1 + 1) * N], U[:, :, b1, :],
                start=True, stop=True,
            )
            ob = outp.tile([N, 2, VO], f32)
            if p % 2 == 0:
                nc.vector.tensor_copy(ob[:], pb[:])
            else:
                nc.scalar.copy(ob[:], pb[:])
            deng = nc.sync if p % 2 == 0 else nc.scalar
            deng.dma_start(out_d[:, b0 : b0 + 2, :], ob[:])

    nc.compile()
    return nc


def _get_nc(mode):
    if mode not in _nc_cache:
        _nc_cache[mode] = _build(mode)
    return _nc_cache[mode]


def _prep_inputs(node_embed, veh_fea, W, b, mode):
    if mode == "bf16":
        import ml_dtypes

        def cast(x):
            return np.ascontiguousarray(x.astype(ml_dtypes.bfloat16))
    else:

        def cast(x):
            return np.ascontiguousarray(x.astype(np.float32))

    node_embed = np.asarray(node_embed, dtype=np.float32)
    veh_fea = np.asarray(veh_fea, dtype=np.float32)
    W = np.asarray(W, dtype=np.float32)
    b = np.asarray(b, dtype=np.float32)

    nodeT = cast(node_embed.transpose(2, 0, 1).reshape(D, B * N))
    vehT = cast(veh_fea.transpose(2, 0, 1).reshape(E, B * V))
    W3 = W.reshape(O, D, E)

    in_maps = []
    for c in range(NCORES):
        sel = slice(c * OS, (c + 1) * OS)
        wt = cast(W3[sel].transpose(2, 0, 1).reshape(E, OS * D))
        in_maps.append({"nodeT": nodeT, "vehT": vehT, "wt": wt})
    return in_maps


def run(node_embed, veh_fea, W, b, trace=False):
    from concourse.bass_utils import run_bass_kernel_spmd

    nc = _get_nc(MODE)
    in_maps = _prep_inputs(node_embed, veh_fea, W, b, MODE)
    res = run_bass_kernel_spmd(nc, in_maps, list(range(NCORES)), trace=trace)
    # per-core out is [N, B, (o,v)] -> [B,N,V,OS]; bias added here (host)
    outs = [
        r["out"].reshape(N, B, OS, V).transpose(1, 0, 3, 2) for r in res.results
    ]
    full = np.concatenate(outs, axis=3) + np.asarray(b, np.float32)
    full = np.ascontiguousarray(full, dtype=np.float32)
    return full, res


def kernel(node_embed, veh_fea, W, b):
    return run(node_embed, veh_fea, W, b)[0]
